# revision 1
# baseline (speedup 1.0000x reference)
import os
import sys

import numpy as np

for _p in ("/opt/trn_rl_repo", "/root/.axon_site/_ro/trn_rl_repo"):
    if os.path.isdir(_p) and _p not in sys.path:
        sys.path.insert(0, _p)

import concourse.tile as tile
from concourse import bacc, mybir

# Problem: y = causal dilated conv1d (C=64->64, K=2, dilation=64) over x[16,64,16384],
# then tanh(y)*sigmoid(y).  Sharded data-parallel over batch: 2 batches per core.
B, C, T = 16, 64, 16384
KERNEL = 2
DIL = 64
N_CORES = 8
B_PER = B // N_CORES  # 2
P = B_PER * C  # 128 partitions: batch 0 on 0..63, batch 1 on 64..127
NT = int(os.environ.get("KERNEL_NT", "2048"))  # time-tile (columns per DMA)
CHUNK = 512  # PSUM bank free size (fp32)
F32 = mybir.dt.float32
# float32r = single-pass "relaxed" fp32 matmul on the PE (vs 2-pass exact fp32)
MM_DTYPE = os.environ.get("KERNEL_MM_DTYPE", "float32r")


def _build_program():
    nc = bacc.Bacc("TRN2", target_bir_lowering=False, debug=False)
    # float32r: same fp32 bits, but typed for the PE's single-pass fp32 mode
    mmdt = getattr(mybir.dt, MM_DTYPE)
    x_in = nc.dram_tensor("x", [B_PER, C, T], mmdt, kind="ExternalInput")
    # Host-preprocessed weights: wt[k] is the 128x128 block-diagonal stationary
    # matrix for tap k (two copies of w[:,:,k].T on the diagonal), so one K=128
    # matmul computes both batches' 64x64 channel mix.
    wt_in = nc.dram_tensor("wt", [KERNEL, P, P], mmdt, kind="ExternalInput")
    y_out = nc.dram_tensor("y", [B_PER, C, T], F32, kind="ExternalOutput")

    x_flat = x_in[:].flatten_outer_dims()  # [128, T]
    y_flat = y_out[:].flatten_outer_dims()  # [128, T]

    with tile.TileContext(nc) as tc:
        with (
            tc.tile_pool(name="wpool", bufs=1) as wpool,
            tc.tile_pool(name="xpool", bufs=9) as xpool,
            tc.tile_pool(name="opool", bufs=12) as opool,
            tc.tile_pool(name="actpool", bufs=4) as actpool,
            tc.tile_pool(
                name="psum",
                bufs=max(2, 4096 // int(os.environ.get("KERNEL_ACT_FD", "1024"))),
                space="PSUM",
            ) as psumpool,
        ):
            # tiny weight loads first (~0.15us each) so the first matmuls are
            # gated only on the first x tile, not on late weight DMAs
            wblk = []
            for k in range(KERNEL):
                wk = wpool.tile([P, P], mmdt, tag=f"w{k}")
                nc.sync.dma_start(out=wk[:], in_=wt_in[k])
                wblk.append(wk)

            # first/last tiles are half-size: the first matmuls wait on a
            # smaller first DMA, and the final act->mul->store drain is shorter
            EDGE = NT // 2
            tiles = (
                [(0, EDGE)]
                + [(EDGE + i * NT, NT) for i in range((T - 2 * EDGE) // NT)]
                + [(T - EDGE, EDGE)]
            )

            xt0 = xpool.tile([P, EDGE + DIL], mmdt, tag="xt")
            nc.vector.memset(xt0[:, 0:DIL].bitcast(F32), 0.0)
            nc.sync.dma_start(out=xt0[:, DIL:], in_=x_flat[:, 0:EDGE])

            # prime the ACT function tables on a dummy element so the ~2.6us
            # of ACT_TABLE_LOADs overlap the first input DMA
            prime = wpool.tile([1, 2], F32, tag="prime")
            nc.vector.memset(prime[:], 0.0)
            nc.scalar.activation(
                out=prime[:, 0:1],
                in_=prime[:, 1:2],
                func=mybir.ActivationFunctionType.Tanh,
            )
            nc.scalar.activation(
                out=prime[:, 0:1],
                in_=prime[:, 1:2],
                func=mybir.ActivationFunctionType.Sigmoid,
            )

            ACT_FD = int(os.environ.get("KERNEL_ACT_FD", "1024"))
            n_tiles = len(tiles)
            for it, (t0, nt) in enumerate(tiles):
                if it == 0:
                    xt = xt0
                else:
                    # x tile carries a DIL-column left halo: col j = t0 - DIL + j
                    xt = xpool.tile([P, nt + DIL], mmdt, tag="xt")
                    nc.sync.dma_start(out=xt[:], in_=x_flat[:, t0 - DIL : t0 + nt])

                # first/last tile use finer blocks: the first output DMA
                # starts sooner and the final act->mul->store drain is shorter
                if it == 0 or it == n_tiles - 1:
                    blocks = [CHUNK] * (nt // CHUNK)
                else:
                    blocks = [ACT_FD] * (nt // ACT_FD)
                base = 0
                for fd in blocks:
                    # y[t] = W1^T @ x[t]  +  W0^T @ x[t-DIL]
                    ps = psumpool.tile([P, fd], F32, tag="ps")
                    for k in (1, 0):
                        for c in range(0, fd, CHUNK):
                            nc.tensor.matmul(
                                out=ps[:, c : c + CHUNK],
                                lhsT=wblk[k][:],
                                rhs=xt[
                                    :,
                                    base + c + k * DIL : base + c + k * DIL + CHUNK,
                                ],
                                start=(k == 1),
                                stop=(k == 0),
                            )
                    th = actpool.tile([P, fd], F32, tag="th")
                    sg = actpool.tile([P, fd], F32, tag="sg")
                    nc.scalar.activation(
                        out=th[:], in_=ps[:], func=mybir.ActivationFunctionType.Tanh
                    )
                    nc.scalar.activation(
                        out=sg[:], in_=ps[:], func=mybir.ActivationFunctionType.Sigmoid
                    )
                    ot = opool.tile([P, fd], F32, tag="ot")
                    nc.vector.tensor_mul(ot[:], th[:], sg[:])
                    # per-block output DMA from gpsimd: stores start as soon
                    # as each block's multiply lands (gpsimd keeps them off
                    # the sync ring, whose FIFO carries the input stream)
                    nc.gpsimd.dma_start(
                        out=y_flat[:, t0 + base : t0 + base + fd], in_=ot[:]
                    )
                    base += fd
    nc.finalize()
    return nc


def _host_weights(w: np.ndarray) -> np.ndarray:
    wt = np.zeros((KERNEL, P, P), dtype=np.float32)
    for k in range(KERNEL):
        wTk = np.ascontiguousarray(w[:, :, k].T)  # [ci, co]
        for b in range(B_PER):
            wt[k, b * C : (b + 1) * C, b * C : (b + 1) * C] = wTk
    return wt


def _ensure_ntff_hook():
    """Recreate the antenv.axon_hooks NTFF profiling shim if the image lacks it."""
    import types

    try:
        import antenv.axon_hooks  # noqa: F401

        return
    except ImportError:
        pass
    import antenv

    mod = types.ModuleType("antenv.axon_hooks")
    _h = {"hook": None}
    mod.set_axon_ntff_profile_hook = lambda h: _h.__setitem__("hook", h)
    mod.get_axon_ntff_profile_hook = lambda: _h["hook"]
    sys.modules["antenv.axon_hooks"] = mod
    antenv.axon_hooks = mod
    try:
        from trn_agent_boot.trn_boot import _ntff_profile_via_ctypes

        hook = _ntff_profile_via_ctypes("/opt/axon/libaxon_pjrt.so")
        if hook is not None:
            mod.set_axon_ntff_profile_hook(hook)
    except Exception as e:  # degrade to no-trace rather than crash
        print(f"ntff hook setup failed: {e}", file=sys.stderr)


def _run_spmd(x: np.ndarray, w: np.ndarray, trace: bool = False):
    from concourse import bass_utils
    from concourse.bass_utils import run_bass_kernel_spmd

    if trace:
        _ensure_ntff_hook()
        bass_utils.upload_artifacts = lambda tmpdir: tmpdir

    nc = _build_program()
    wt = _host_weights(w)
    in_maps = [
        {"x": np.ascontiguousarray(x[i * B_PER : (i + 1) * B_PER]), "wt": wt}
        for i in range(N_CORES)
    ]
    kwargs = {}
    if trace:
        import tempfile

        os.makedirs("/tmp/kernel_trace", exist_ok=True)
        kwargs["tmpdir"] = tempfile.mkdtemp(dir="/tmp/kernel_trace")
    res = run_bass_kernel_spmd(nc, in_maps, list(range(N_CORES)), trace=trace, **kwargs)
    y = np.concatenate([res.results[i]["y"] for i in range(N_CORES)], axis=0)
    return y, res


def kernel(x: np.ndarray, w: np.ndarray) -> np.ndarray:
    x = np.ascontiguousarray(np.asarray(x, dtype=np.float32))
    w = np.ascontiguousarray(np.asarray(w, dtype=np.float32))
    trace = os.environ.get("KERNEL_TRACE", "0") == "1"
    y, res = _run_spmd(x, w, trace=trace)
    if trace:
        global LAST_RESULTS
        LAST_RESULTS = res
    return y


LAST_RESULTS = None

